# revision 40
# baseline (speedup 1.0000x reference)
"""Multi-head attention on 8 TRN2 NeuronCores — v9 (final).

Problem: queries [B,N,L,H,E], keys [B,N,S,H,E], values [B,N,S,H,D]
         out[b,n,l,h,:] = softmax(Q[b,n,l,h,:] @ K[b,n,:,h,:]^T / sqrt(E)) @ V[b,n,:,h,:]
with B,N,L,S,H,E,D = 4,7,512,512,8,64,64.

Sharding: head-parallel — core c computes all B*N=28 (b,n) slices for head h=c.

The steady state is bound by PSUM->SBUF egress on ScalarE+DVE (every attn
element must exit PSUM through one of them at ~1 elem/cycle/lane; GPSIMD
and DMA cannot read PSUM). v6 balances the two engines:
  1. QK row-tiled 2x concurrent: Q^T duplicated in both partition halves;
     K chunks c0/c1 in rows 0-63, c2/c3 in rows 64-127 -> 4 score chunks
     [128s, 512l] in ~2x512 column-cycles.
  2. exp 3-way split per slice (2048+260 egress cols balanced):
       ScalarE: exact-exp ACTIVATE on psA[:, 0:1408]       (~1.43us)
       DVE:     Schraudolph on psA[:, 1408:1536] (sliver), (~0.26us)
                Schraudolph on psB [128,512],              (~0.68us)
                po copy PSUM->SBUF [128,260] fp16          (~0.43us)
     Schraudolph: i16 = rne(score*EA + EB), bits as fp16 ~= exp(score/8);
     exact because SCALE = 1/sqrt(64) = 1/8.
  3. PV attn-stationary: po[128l, 65] per l-chunk accumulates
     lhsT=attnT[sc, lc*128:+128], rhs=[V_sc | ones] (FD=65). Column 64 is
     the softmax denominator -> output partition dim = l, only 260 egress
     cols for the po copy.
  4. po [128, 260] -> SBUF fp16; out-DMA issued per pair immediately
     (spreads HBM writes across the kernel instead of a tail pileup);
     host does the final transpose + divide.

PSUM budget (8 banks): psA [128,1536]x2 (6) + psB [128,512] (1)
 + po [128,260] (1) = 8. Last slice's po is carved from the previous
slice's psA bank (no later consumer) to shorten the drain.

Emission order per slice: QK(k); PV+copy(k-1); exp(k) — the in-order DVE
queue then runs [po-copy(k-1); TS(k)] so the po bank frees before PV(k)
needs it. Head: staggered prefetch + fine-split first DMA + 5 warm MMs
(HAM clock gate) + split first ACT. Tail: split last ACTs/copies; final
DMAs issued from the then-idle ScalarE HWDGE queue.

Steady state measures ~1.4us/slice with ScalarE 100% busy and DVE ~99%
— the PSUM-egress floor for this dataflow (2308 cols/slice over two
engines at ~1 elem/cycle/lane). Typical exec ~57-59.5us; occasional
~72us runs are chip-level P0/thermal throttling (steady uniformly
~21% slower, also seen on the unmodified baseline).
"""

import numpy as np

B, N, L, S, H, E, D = 4, 7, 512, 512, 8, 64, 64
NS = B * N          # 28 slices per core
NP = NS // 2        # 14 slice-pairs (DMA granularity)
P = 128
SC = S // P         # 4 s-chunks
LC = L // P         # 4 l-chunks
SCALE = 1.0 / float(np.sqrt(E))
YS = 128            # exp sliver (chunk 2, l-block 3) done on DVE, not ScalarE

QOFF, KOFF, VOFF = 0, 512, 768
SLC = 1028          # cols per slice
EA = 1024.0 * float(np.log2(np.e)) / 8.0
EB = 15360.0 - 60.0

_CACHE = {}


def _build_program():
    import concourse.mybir as mybir
    import concourse.tile as tile
    from concourse import bacc
    import concourse.bass as bass

    f32 = mybir.dt.float32
    f16 = mybir.dt.float16
    i16 = mybir.dt.int16
    Exp = mybir.ActivationFunctionType.Exp
    MUL = mybir.AluOpType.mult
    ADD = mybir.AluOpType.add

    nc = bacc.Bacc("TRN2", target_bir_lowering=False, debug=False)
    inp = nc.dram_tensor("inp", [NP, P, 2 * SLC], f16, kind="ExternalInput").ap()
    o = nc.dram_tensor("o", [NP, P, 2 * 260], f16, kind="ExternalOutput").ap()

    with tile.TileContext(nc) as tc:
        with (
            tc.tile_pool(name="inpool", bufs=1) as in_pool,
            tc.tile_pool(name="attnA", bufs=1) as aA_pool,
            tc.tile_pool(name="attnB", bufs=1) as aB_pool,
            tc.tile_pool(name="attnS", bufs=1) as aS_pool,
            tc.tile_pool(name="osb", bufs=1) as osb_pool,
            tc.tile_pool(name="psA", bufs=1, space=bass.MemorySpace.PSUM) as psA_pool,
            tc.tile_pool(name="psB", bufs=1, space=bass.MemorySpace.PSUM) as psB_pool,
            tc.tile_pool(name="po", bufs=1, space=bass.MemorySpace.PSUM) as po_pool,
        ):
            # --- warm-up ---------------------------------------------------
            # HAM: back-to-back dummy matmuls open the PE clock gate
            # (1.2 -> 2.4 GHz) before the steady-state pipeline begins.
            warm = in_pool.tile([P, L], f16, tag="warm")
            nc.vector.memset(warm[:], 1.0)
            # Preload ScalarE exp table (~2.7us once) with a tiny ACTIVATE.
            dummyA = osb_pool.tile([1, 8], f32, tag="dumA")
            nc.scalar.activation(dummyA[:], warm[0:1, 0:8], Exp, scale=SCALE)
            # Preload DVE tensor_scalar path.
            dummyB = aB_pool.tile([1, 8], i16, tag="dumB")
            nc.vector.tensor_scalar(dummyB[:], warm[0:1, 0:8], EA, EB, MUL, ADD)
            wps = psB_pool.tile([P, L], f32, tag="b0")
            for _ in range(4):
                nc.tensor.matmul(
                    wps[:], lhsT=warm[:, 0:P], rhs=warm[:], start=True, stop=True
                )

            # --- input DMA, 4 pairs ahead ---------------------------------
            in_tiles = {}

            def load_pair(p, split=False, eng=None):
                if p < NP and p not in in_tiles:
                    t = in_pool.tile([P, 2 * SLC], f16, tag=f"t{p % 5}", name=f"in{p}")
                    eng = eng or nc.sync
                    if split:
                        # land the pair's first slice's QK operands first,
                        # then its VO, then the second slice — its first QK
                        # starts as early as possible (all 8 cores contend
                        # for HBM in the prologue, so transfers are slow;
                        # keep the critical piece small)
                        eng.dma_start(t[:, 0:VOFF], inp[p][:, 0:VOFF])
                        eng.dma_start(t[:, VOFF:SLC], inp[p][:, VOFF:SLC])
                        eng.dma_start(t[:, SLC:2 * SLC], inp[p][:, SLC:2 * SLC])
                    else:
                        eng.dma_start(t[:], inp[p])
                    in_tiles[p] = t

            # Prefetch pairs 0-1 with transfers ordered by CONSUMPTION order
            # (the prologue burst is slow — ~50-200GB/s while all 8 cores
            # collide — so arrival order is everything): slice-0 QK first,
            # then slice-2's half (previously it sat behind all of pair 0
            # and slices 2-3 started ~2.7us late), then the slack pieces.
            t0 = in_pool.tile([P, 2 * SLC], f16, tag="t0", name="in0")
            t1 = in_pool.tile([P, 2 * SLC], f16, tag="t1", name="in1")
            # strict consumption order: QK_0 < QK_1 (11.3us) < PV_0's VO
            # (12.3) < QK_2 (12.7) < QK_3 (14.1); pair-1 halved so slice 2's
            # completion sem fires without waiting for slice 3's bytes.
            # (Halving pair 2 as well measured neutral-to-worse: it delays
            # the pair-3 issue and the prologue burst rate is too noisy to
            # reward the finer granularity.)
            nc.sync.dma_start(t0[:, 0:VOFF], inp[0][:, 0:VOFF])        # s0 QK
            nc.sync.dma_start(t0[:, SLC:2 * SLC], inp[0][:, SLC:2 * SLC])  # s1
            nc.sync.dma_start(t0[:, VOFF:SLC], inp[0][:, VOFF:SLC])    # s0 VO
            nc.sync.dma_start(t1[:, 0:SLC], inp[1][:, 0:SLC])          # s2
            nc.sync.dma_start(t1[:, SLC:2 * SLC], inp[1][:, SLC:2 * SLC])  # s3
            in_tiles[0] = t0
            in_tiles[1] = t1

            def emit_qk(k, in_t, j):
                """Score chunks for slice k. j = slice's half of the pair tile."""
                q2 = in_t[:, j * SLC + QOFF: j * SLC + QOFF + L]
                k2 = in_t[:, j * SLC + KOFF: j * SLC + KOFF + 2 * P]
                psA = psA_pool.tile([P, 3 * L], f32, tag=f"A{k % 2}")
                psB = psB_pool.tile([P, L], f32, tag="b0")
                # unit 0: chunk0 (rows 0-63) || chunk2 (rows 64-127)
                nc.tensor.matmul(psA[:, 0:L], lhsT=k2[0:E, 0:P], rhs=q2[0:E, :],
                                 start=True, stop=True)
                nc.tensor.matmul(psA[:, 2 * L:3 * L], lhsT=k2[E:P, 0:P], rhs=q2[E:P, :],
                                 start=True, stop=True)
                # unit 1: chunk1 || chunk3
                nc.tensor.matmul(psA[:, L:2 * L], lhsT=k2[0:E, P:2 * P], rhs=q2[0:E, :],
                                 start=True, stop=True)
                nc.tensor.matmul(psB[:], lhsT=k2[E:P, P:2 * P], rhs=q2[E:P, :],
                                 start=True, stop=True)
                return psA, psB

            def emit_exp(k, psA, psB):
                aA = aA_pool.tile([P, 3 * L - YS], f16, tag=f"A{k % 3}")
                if k >= NS - 2 or k == 0:
                    # head/tail drain: split so the consumer can start while
                    # the rest of the exp still runs (k=0: chunk 0 is ready
                    # after the first QK matmul; tail: the last PV's early
                    # matmuls can start under the remaining exp)
                    sp = L if k == 0 else 2 * L
                    nc.scalar.activation(aA[:, 0:sp], psA[:, 0:sp], Exp, scale=SCALE)
                    nc.scalar.activation(aA[:, sp:3 * L - YS], psA[:, sp:3 * L - YS],
                                         Exp, scale=SCALE)
                else:
                    nc.scalar.activation(aA[:], psA[:, 0:3 * L - YS], Exp, scale=SCALE)
                # DVE: Schraudolph on the sliver + chunk 3 (exp(x/8) exact
                # since SCALE = 1/8)
                aS = aS_pool.tile([P, YS], i16, tag=f"S{k % 3}")
                nc.vector.tensor_scalar(aS[:], psA[:, 3 * L - YS:3 * L], EA, EB, MUL, ADD)
                aB = aB_pool.tile([P, L], i16, tag=f"B{k % 3}")
                nc.vector.tensor_scalar(aB[:], psB[:], EA, EB, MUL, ADD)
                return aA, aS, aB

            psA_tiles = {}

            def emit_pv(k, in_t, j, psA, aA, aS, aB):
                """Attn-stationary PV: po[128l, 65] per l-chunk accumulates
                over the 4 s-chunks; col 64 = softmax denominator."""
                f16aS = aS[:].bitcast(f16)
                f16aB = aB[:].bitcast(f16)
                vo = in_t[:, j * SLC + VOFF: j * SLC + VOFF + SC * 65]
                if k == NS - 1:
                    # carve the last po out of the previous slice's psA bank
                    # (already consumed; no later QK reuses it)
                    po = psA_tiles[NS - 2][:, 0:LC * 65]
                else:
                    pot = po_pool.tile([P, LC * 65], f32, tag="po0", name="pot")
                    po = pot[:]
                for lc in range(LC):
                    for sc in range(SC):
                        if sc < 3:
                            if sc == 2 and lc == 3:
                                st = f16aS[:]
                            else:
                                st = aA[:, sc * L + lc * P: sc * L + (lc + 1) * P]
                        else:
                            st = f16aB[:, lc * P:(lc + 1) * P]
                        nc.tensor.matmul(
                            po[:, lc * 65:(lc + 1) * 65],
                            lhsT=st,
                            rhs=vo[:, sc * 65:(sc + 1) * 65],
                            start=(sc == 0),
                            stop=(sc == SC - 1),
                        )
                return po

            osb_tiles = {}

            def emit_tail(kk, it, jj, psA, aA, aS, aB):
                po = emit_pv(kk, it, jj, psA, aA, aS, aB)
                pp = kk // 2
                if pp not in osb_tiles:
                    osb_tiles[pp] = osb_pool.tile([P, 2 * 260], f16, tag=f"o{pp % 3}", name=f"osb{pp}")
                osb = osb_tiles[pp]
                off = (kk % 2) * 260
                half = slice(off, off + 260)
                if kk == NS - 1:
                    # finest-grained drain for the very last slice: copy and
                    # DMA each po half as soon as its PV l-chunks finish.
                    # Issue these DMAs from the Scalar engine's HWDGE queue —
                    # Scalar is idle after its last ACT (these are emitted
                    # after it), the queue is empty, and the transfers start
                    # the instant each copy lands instead of queueing behind
                    # the Sync engine's issue stream.
                    # halves go to the two (now idle) HWDGE queues so the
                    # ~0.7us descriptor-gen issues run concurrently
                    nc.vector.tensor_copy(osb[:, off:off + 130], po[:, 0:130])
                    nc.scalar.dma_start(o[pp][:, off:off + 130], osb[:, off:off + 130])
                    nc.vector.tensor_copy(osb[:, off + 130:off + 260], po[:, 130:260])
                    nc.sync.dma_start(o[pp][:, off + 130:off + 260], osb[:, off + 130:off + 260])
                    del osb_tiles[pp]
                    return
                nc.vector.tensor_copy(osb[:, half], po)
                if pp == NP - 1:
                    # tail drain: DMA each half out as soon as its copy lands
                    nc.sync.dma_start(o[pp][:, half], osb[:, half])
                    if kk % 2 == 1:
                        del osb_tiles[pp]
                elif kk % 2 == 1:
                    # issue the pair's out-DMA immediately: spreads HBM writes
                    # across the kernel instead of piling them up at the end
                    # (4-pair input prefetch gives the in-order Sync queue
                    # plenty of slack to absorb the wait-for-copy)
                    nc.sync.dma_start(o[pp], osb[:])
                    del osb_tiles[pp]

            pend = []
            for k in range(NS):
                pair, j = k // 2, k % 2
                in_t = in_tiles[pair]
                if k == 1:
                    load_pair(2)
                    load_pair(3)
                elif j == 1 and k >= 3:
                    load_pair((k - 3) // 2 + 4)
                psA, psB = emit_qk(k, in_t, j)
                psA_tiles[k] = psA
                # emit PV/copy of slice k-1 BEFORE slice k's exp ops: the
                # in-order DVE queue then runs [po-copy(k-1); TSs(k)], so the
                # po bank is free well before PV(k) needs it (previously the
                # copy trailed both TSs and finished right at the edge of
                # ACT(k), alternately stalling every other PV)
                if pend:
                    emit_tail(*pend.pop(0))
                attn = emit_exp(k, psA, psB)
                pend.append((k, in_t, j, psA) + attn)
            while pend:
                emit_tail(*pend.pop(0))
    nc.compile()
    return nc


def _prep_inputs(queries, keys, values):
    """Pack per-core fp16 inputs. Core c gets head h=c."""
    q = np.asarray(queries, dtype=np.float32)
    k = np.asarray(keys, dtype=np.float32)
    v = np.asarray(values, dtype=np.float32)

    # Q^T / K^T per slice: [H, NS, E, L]
    qt = np.ascontiguousarray(q.transpose(3, 0, 1, 4, 2)).reshape(H, NS, E, L)
    kt = np.ascontiguousarray(k.transpose(3, 0, 1, 4, 2)).reshape(H, NS, E, S)
    q2 = np.concatenate([qt, qt], axis=2)                     # [H, NS, 128, 512]
    ktc = kt.reshape(H, NS, E, SC, P)
    k2 = np.concatenate(                                       # [H, NS, 128, 256]
        [
            ktc[:, :, :, 0:2].reshape(H, NS, E, 2 * P),
            ktc[:, :, :, 2:4].reshape(H, NS, E, 2 * P),
        ],
        axis=2,
    )
    # VO: [H, NS, sc, s, 65] = [V | ones] -> [H, NS, 128, 260]
    vt = v.transpose(3, 0, 1, 2, 4).reshape(H, NS, SC, P, D)
    vo = np.ones((H, NS, SC, P, 65), dtype=np.float32)
    vo[..., 0:D] = vt
    vo = np.ascontiguousarray(vo.transpose(0, 1, 3, 2, 4)).reshape(H, NS, P, SC * 65)

    inp = np.concatenate([q2, k2, vo], axis=3).astype(np.float16)  # [H, NS, 128, 1028]
    inp = np.ascontiguousarray(
        inp.reshape(H, NP, 2, P, SLC).transpose(0, 1, 3, 2, 4)
    ).reshape(H, NP, P, 2 * SLC)
    return [{"inp": inp[c]} for c in range(H)]


def _run(in_maps, trace=False, tmpdir=None):
    from concourse.bass_utils import run_bass_kernel_spmd

    if "nc" not in _CACHE:
        _CACHE["nc"] = _build_program()
    kwargs = {}
    if tmpdir is not None:
        kwargs["tmpdir"] = tmpdir
    return run_bass_kernel_spmd(
        _CACHE["nc"], in_maps, core_ids=list(range(H)), trace=trace, **kwargs
    )


def kernel(queries, keys, values, _trace=False, _results_out=None, _tmpdir=None):
    in_maps = _prep_inputs(queries, keys, values)
    res = _run(in_maps, trace=_trace, tmpdir=_tmpdir)
    if _results_out is not None:
        _results_out.append(res)
    # res.results[c]["o"]: [NP, 128, 520] -> [NS, 128, 260]
    raw = np.stack([res.results[c]["o"] for c in range(H)], axis=0).astype(np.float32)
    raw = raw.reshape(H, NP, P, 2, 260).transpose(0, 1, 3, 2, 4).reshape(H, NS, P, LC, 65)
    num = raw[..., 0:D]            # [H, NS, p, lc, D]
    den = raw[..., D:D + 1]
    out = num / den                # [H, NS, p, lc, D]
    # l = lc*128 + p -> axes (NS, lc, p, H, D) then merge (lc, p) -> L
    out = out.transpose(1, 3, 2, 0, 4).reshape(B, N, L, H, D)
    return np.ascontiguousarray(out.astype(np.float32))
